# revision 27
# baseline (speedup 1.0000x reference)
"""Multi-head attention (B=4, S=2048, D=1024, H=16, dk=64) on 8 trn2 cores.

Sharding: core c = (batch b = c//2, head-group g = c%2). Each core computes
its batch's QKV projections restricted to its 8 heads (512 output dims),
runs attention for those heads, and produces a partial out-projection
y_partial = ctx_g @ Wo[:, g*512:(g+1)*512].T  of shape [S, D].
Host: y[b] = y_partial[b,0] + y_partial[b,1] + bo.

The mask input is ignored: the problem spec pins mask to all-ones
(fill="ones"), making the masking a no-op.

v6 design (v4 + software-pipelined attention):
  - ALL layout work is done on the host inside kernel(): inputs arrive in
    DRAM already bf16 and pre-transposed (xqT/xkT/xvT = x.T [D,S],
    wqT/wkT/wvT = W_g.T [D,EG], woT = Wo[:,g].T [EG,D]). No on-device
    casts, no transpose DMAs. ScalarE runs ONLY the exp stream (measured
    ~1.25us per [128,1024] exp -> ~320us/core floor; in-situ the
    dependency spacing hides part of the access bubbles and the attention
    phase alone measures ~294us); DVE does bias adds, evictions, and the
    on-chip reciprocal broadcast (STREAM_SHUFFLE).
  - attention processes head PAIRS: the two K=64 score matmuls of a pair
    auto-derive tile_position (0,0)/(64,0) from their base partitions and
    run CONCURRENTLY in the PE array (row tiling) - 2x score throughput.
    Both heads' scores for an sq-chunk of 512 land in one [128,1024] PSUM
    tile, consumed by a single 1024-wide exp.
  - the PV matmuls are SOFTWARE-PIPELINED one 2-skt batch behind the
    scores/exp stream, so every PV (and pumped projection) matmul is
    wait-free when the PE reaches it and its LDWEIGHTS can prefetch; the
    only blocking PE waits are the score slots' sc-buf releases.
  - PV keeps the ones-column trick: vh per head is [sk,65], row 64 of the
    ctx accumulator is the softmax denominator (M=65 rides free).
  - projections are PUMPED into the PE stream as in v4 (v-projection
    st-pairs as two bank-interleaved accumulation chains during chunk 0,
    q/k/out 2-way j-interleaved). NOTE: accumulation chains cannot be
    interleaved at sub-bank granularity - a chain's start=True zeroes the
    whole PSUM bank region, so at most one chain per 512-f32 bank.

PSUM plan (8 banks): scores/vproj [128,1024] x2 bufs (4) + ctx 2x[65,512]
(2) + proj/outproj pj [128,1024] (2).
"""

import sys

if "/opt/trn_rl_repo" not in sys.path:
    sys.path.insert(0, "/opt/trn_rl_repo")

import numpy as np

B = 4
S = 2048
D = 1024
H_TOTAL = 16
DK = 64
NCORES = 8
EG = 512          # per-core head-group width (8 heads x 64)
HPC = EG // DK    # heads per core = 8
P = 128
NPAIR = HPC // 2  # 4 head pairs per core
SQC = 512         # per-head sq chunk width in attention
NSQC = S // SQC   # 4
NSKT = S // P     # 16 sk chunks

_CACHE: dict = {}


def _build_module(loop_n=None, parts="all", loads_outside=False):
    import itertools
    import concourse.bacc as bacc
    import concourse.tile as tile
    import concourse.mybir as mybir
    import concourse.bass as bass
    import contextlib

    dt = mybir.dt
    f32, bf16 = dt.float32, dt.bfloat16
    AF = mybir.ActivationFunctionType

    nc = bacc.Bacc("TRN2", debug=False, num_devices=NCORES, num_swdge_queues=4)

    # ---- DRAM I/O (host-prepped: bf16, pre-transposed) ----
    xqT = nc.dram_tensor("xqT", [D, S], bf16, kind="ExternalInput").ap()
    xkT = nc.dram_tensor("xkT", [D, S], bf16, kind="ExternalInput").ap()
    xvT = nc.dram_tensor("xvT", [D, S], bf16, kind="ExternalInput").ap()
    wqT = nc.dram_tensor("wqT", [D, EG], bf16, kind="ExternalInput").ap()
    wkT = nc.dram_tensor("wkT", [D, EG], bf16, kind="ExternalInput").ap()
    wvT = nc.dram_tensor("wvT", [D, EG], bf16, kind="ExternalInput").ap()
    woT = nc.dram_tensor("woT", [EG, D], bf16, kind="ExternalInput").ap()
    bq = nc.dram_tensor("bq", [EG], f32, kind="ExternalInput").ap()
    bk = nc.dram_tensor("bk", [EG], f32, kind="ExternalInput").ap()
    bv = nc.dram_tensor("bv", [EG], f32, kind="ExternalInput").ap()
    yp = nc.dram_tensor("yp", [S, D], f32, kind="ExternalOutput").ap()

    with tile.TileContext(nc) as tc:
        with contextlib.ExitStack() as ctx:
            persist = ctx.enter_context(tc.tile_pool(name="persist", bufs=1))
            xv_pool = ctx.enter_context(tc.tile_pool(name="xv", bufs=16))
            ptmp_pool = ctx.enter_context(tc.tile_pool(name="ptmp", bufs=2))
            att_pool = ctx.enter_context(tc.tile_pool(name="att", bufs=5))
            cxs_pool = ctx.enter_context(tc.tile_pool(name="cxs", bufs=2))
            y_pool = ctx.enter_context(tc.tile_pool(name="yout", bufs=2))
            psum = ctx.enter_context(tc.tile_pool(name="ps", bufs=1, space="PSUM"))

            # ---------- persistent SBUF ----------
            wq_sb = [persist.tile([P, EG], bf16, name=f"wq{i}", tag=f"wq{i}")
                     for i in range(8)]
            wk_sb = [persist.tile([P, EG], bf16, name=f"wk{i}", tag=f"wk{i}")
                     for i in range(8)]
            wv_sb = [persist.tile([P, EG], bf16, name=f"wv{i}", tag=f"wv{i}")
                     for i in range(8)]
            wo_sb = [persist.tile([P, D], bf16, name=f"wo{i}", tag=f"wo{i}")
                     for i in range(4)]
            xq_sb = [persist.tile([P, S], bf16, name=f"xq{i}", tag=f"xq{i}")
                     for i in range(8)]
            xk_sb = [persist.tile([P, S], bf16, name=f"xk{i}", tag=f"xk{i}")
                     for i in range(8)]
            # xv group tiles: [d-128, 4-st-chunk 512] per (group, dc); filled
            # by load_all into a 16-slot rotating pool (2 groups in flight)
            xv_sb = [[None] * 8 for _ in range(4)]
            qhT = [persist.tile([P, S], bf16, name=f"qhT{i}", tag=f"qhT{i}")
                   for i in range(NPAIR)]
            khT = [persist.tile([P, S], bf16, name=f"khT{i}", tag=f"khT{i}")
                   for i in range(NPAIR)]
            vh = [persist.tile([P, HPC * (DK + 1)], bf16, name=f"vh{i}",
                               tag=f"vh{i}") for i in range(NSKT)]
            ctxT = [persist.tile([P, S], bf16, name=f"ctxT{i}", tag=f"ctxT{i}")
                    for i in range(NPAIR)]

            # biases (gpsimd: strided/broadcast APs need SWDGE)
            bq_sb = persist.tile([P, NPAIR], f32, tag="bq_sb")
            bk_sb = persist.tile([P, NPAIR], f32, tag="bk_sb")
            bv_sb = persist.tile([P, EG], f32, tag="bv_sb")
            recB = persist.tile([DK, SQC], f32, tag="recB")
            nc.vector.memset(recB[:], 0.0)
            zero_col = persist.tile([P, 1], f32, tag="zero_col")
            nc.vector.memset(zero_col[:], 0.0)
            nc.gpsimd.dma_start(
                out=bq_sb[:],
                in_=bass.AP(tensor=bq.tensor, offset=bq.offset,
                            ap=[[1, P], [P, NPAIR]]))
            nc.gpsimd.dma_start(
                out=bk_sb[:],
                in_=bass.AP(tensor=bk.tensor, offset=bk.offset,
                            ap=[[1, P], [P, NPAIR]]))
            nc.gpsimd.dma_start(
                out=bv_sb[:],
                in_=bass.AP(tensor=bv.tensor, offset=bv.offset,
                            ap=[[0, P], [1, EG]]))

            def load_all():
                # queue plan: scalar(Act) = xq staging (done long before the
                # exp stream claims ScalarE); sync(SP) = wv, xv g0/g1, xk,
                # xv g2/g3, then y stores later; gpsimd(SWDGE) = biases +
                # wq/wk/wo. First halves (sh0) of xq/xk land first so the
                # prep-head projections can start before the full reload.
                for dc in range(8):
                    nc.scalar.dma_start(out=xq_sb[dc][:, 0:1024],
                                        in_=xqT[dc * P:(dc + 1) * P, 0:1024])
                for dc in range(8):
                    nc.scalar.dma_start(out=xq_sb[dc][:, 1024:2048],
                                        in_=xqT[dc * P:(dc + 1) * P,
                                                1024:2048])
                for dc in range(8):
                    nc.sync.dma_start(out=wv_sb[dc][:],
                                      in_=wvT[dc * P:(dc + 1) * P, :])
                for g in range(2):
                    for dc in range(8):
                        xt = xv_pool.tile([P, 4 * P], bf16, name="xvt",
                                          tag="xvt")
                        nc.sync.dma_start(
                            out=xt[:],
                            in_=xvT[dc * P:(dc + 1) * P,
                                    g * 4 * P:(g + 1) * 4 * P])
                        xv_sb[g][dc] = xt
                for dc in range(8):
                    nc.sync.dma_start(out=xk_sb[dc][:, 0:1024],
                                      in_=xkT[dc * P:(dc + 1) * P, 0:1024])
                for dc in range(8):
                    nc.sync.dma_start(out=xk_sb[dc][:, 1024:2048],
                                      in_=xkT[dc * P:(dc + 1) * P,
                                              1024:2048])
                for g in range(2, 4):
                    if loads_outside:
                        break
                    for dc in range(8):
                        xt = xv_pool.tile([P, 4 * P], bf16, name="xvt",
                                          tag="xvt")
                        nc.sync.dma_start(
                            out=xt[:],
                            in_=xvT[dc * P:(dc + 1) * P,
                                    g * 4 * P:(g + 1) * 4 * P])
                        xv_sb[g][dc] = xt
                for dc in range(8):
                    nc.gpsimd.dma_start(out=wq_sb[dc][:],
                                        in_=wqT[dc * P:(dc + 1) * P, :])
                    nc.gpsimd.dma_start(out=wk_sb[dc][:],
                                        in_=wkT[dc * P:(dc + 1) * P, :])
                for pc in range(4):
                    nc.gpsimd.dma_start(out=wo_sb[pc][:],
                                        in_=woT[pc * P:(pc + 1) * P, :])

            # ---------- projections ----------
            # Full-K matmuls with stationary reuse: each 128x128 stationary
            # is streamed against two 512-wide moving slices (LDWEIGHTS
            # amortized) - measured ~209ns/MM vs 387 without reuse. K-split
            # row-pairs do NOT overlap inside accumulation chains (measured),
            # so they are not used.
            def v_proj_st2(st_a, st_b, pj=None):
                # two st-chunks of the v projection as INTERLEAVED
                # accumulation chains into the two banks of one [128,1024]
                # PSUM tile: each chain's drain/LDWEIGHTS latency hides under
                # the other chain's matmul (in-chain back-to-back MMs do not
                # pipeline; alternating chains do).
                if pj is None:
                    ps = psum.tile([P, 2 * SQC], f32, name="pv", tag="sc",
                                   bufs=2)
                else:
                    ps = pj
                regs = {st_a: ps[:, 0:EG], st_b: ps[:, EG:2 * EG]}
                for dc in range(8):
                    for st in (st_a, st_b):
                        g, st4 = st // 4, st % 4
                        if loads_outside:
                            g = g % 2  # timing-only alias
                        nc.tensor.matmul(
                            regs[st],
                            lhsT=xv_sb[g][dc][:, st4 * P:(st4 + 1) * P],
                            rhs=wv_sb[dc][:],
                            start=(dc == 0), stop=(dc == 7))
                for st in (st_a, st_b):
                    vt = vh[st].rearrange("p (h c) -> p h c", c=DK + 1)
                    nc.vector.memset(vt[:, :, DK:DK + 1], 1.0)
                    nc.vector.tensor_add(
                        out=vt[:, :, 0:DK],
                        in0=regs[st].rearrange("p (h c) -> p h c", c=DK),
                        in1=bv_sb[:].rearrange("p (h c) -> p h c", c=DK))

            def proj_qk_sh(pair, which, sh):
                # one s-half (2 quarters) of q or k projection for one pair;
                # yields after each matmul (pumpable).
                wsb, xsb, bias = ((wq_sb, xq_sb, bq_sb) if which == "q"
                                  else (wk_sb, xk_sb, bk_sb))
                out_tiles = qhT if which == "q" else khT
                pj = psum.tile([P, 2 * SQC], f32, name="pj", tag="pj")
                for dc in range(8):
                    for j in range(2):
                        nc.tensor.matmul(
                            pj[:, j * SQC:(j + 1) * SQC],
                            lhsT=wsb[dc][:, pair * P:(pair + 1) * P],
                            rhs=xsb[dc][:, sh * 1024 + j * SQC:
                                        sh * 1024 + (j + 1) * SQC],
                            start=(dc == 0), stop=(dc == 7))
                        yield
                nc.vector.tensor_scalar_add(
                    out=out_tiles[pair][:, sh * 1024:(sh + 1) * 1024],
                    in0=pj[:],
                    scalar1=bias[:, pair:pair + 1])
                yield
                # filler slots: let the bias-add retire before the next
                # s-half's first matmul reuses pj
                yield
                yield

            def pair_proj(pair):
                for which in ("q", "k"):
                    for sh in range(2):
                        yield from proj_qk_sh(pair, which, sh)

            # ---------- out-projection ----------
            def outproj_gen(st_list):
                for st in st_list:
                    y_sb = y_pool.tile([P, D], f32, name="y", tag="y")
                    pso = psum.tile([P, 2 * SQC], f32, name="op", tag="pj")
                    for pc in range(4):
                        for ec in range(2):
                            nc.tensor.matmul(
                                pso[:, ec * SQC:(ec + 1) * SQC],
                                lhsT=ctxT[pc][:, st * P:(st + 1) * P],
                                rhs=wo_sb[pc][:, ec * SQC:(ec + 1) * SQC],
                                start=(pc == 0), stop=(pc == 3))
                            yield
                    nc.vector.tensor_copy(out=y_sb[:], in_=pso[:])
                    nc.sync.dma_start(out=yp[st * P:(st + 1) * P, :],
                                      in_=y_sb[:])
                    yield
                    yield

            # ---------- attention ----------
            _SENT = object()

            def attention_chunk(pair, sqc, pump=None, pump_rate=4,
                                pump_per_skt=None):
                # Software-pipelined: the PV matmuls of batch j-1 are issued
                # AFTER the scores+exp of batch j's first skt, so when the PE
                # reaches a PV (or pump) matmul its et input is a full batch
                # old - the instruction is wait-free and its LDWEIGHTS can
                # prefetch into the background weight buffer. The only
                # blocking waits on the PE stream are the two score slots
                # (on the sc-buf release by the exp two slots ago), which in
                # a saturated exp stream cost only the sem-arrival latency.
                q0 = sqc * SQC
                cx = [psum.tile([DK + 1, SQC], f32, name=f"cx{hh}",
                                tag=f"cx{hh}") for hh in range(2)]

                def emit_scores_exp(skt):
                    ps = psum.tile([P, 2 * SQC], f32, name="sc", tag="sc",
                                   bufs=2)
                    for hh in range(2):
                        rsl = slice(hh * DK, (hh + 1) * DK)
                        nc.tensor.matmul(
                            ps[:, hh * SQC:(hh + 1) * SQC],
                            lhsT=khT[pair][rsl, skt * P:(skt + 1) * P],
                            rhs=qhT[pair][rsl, q0:q0 + SQC],
                            start=True, stop=True)
                    et = att_pool.tile([P, 2 * SQC], bf16, name="et",
                                       tag="et")
                    nc.scalar.activation(out=et[:], in_=ps[:],
                                         func=AF.Exp, scale=0.125)
                    return et

                def emit_pv(skt, et):
                    if pump_per_skt is not None:
                        pump_per_skt(skt)
                    for hh in range(2):
                        h = pair * 2 + hh
                        vsl = slice(h * (DK + 1), h * (DK + 1) + DK + 1)
                        nc.tensor.matmul(
                            cx[hh][:],
                            lhsT=vh[skt][:, vsl],
                            rhs=et[:, hh * SQC:(hh + 1) * SQC],
                            start=(skt == 0), stop=(skt == NSKT - 1))

                def do_pump(n):
                    if pump is not None:
                        for _ in range(n):
                            if next(pump, _SENT) is _SENT:
                                break

                pend = None
                for skt2 in range(NSKT // 2):
                    et0 = emit_scores_exp(2 * skt2)
                    if pend is not None:
                        b, p0, p1 = pend
                        emit_pv(b, p0)
                        emit_pv(b + 1, p1)
                    et1 = emit_scores_exp(2 * skt2 + 1)
                    if pend is not None:
                        do_pump(pump_rate)
                    pend = (2 * skt2, et0, et1)
                b, p0, p1 = pend
                emit_pv(b, p0)
                emit_pv(b + 1, p1)
                do_pump(pump_rate)
                # evict PSUM fast, then normalize from SBUF. The reciprocal
                # of the denominator row is broadcast across 64 partitions
                # on-chip: seed both quadrant heads, then STREAM_SHUFFLE with
                # an all-zeros mask replicates partition 0 of each quadrant.
                for hh in range(2):
                    cxs = cxs_pool.tile([DK + 1, SQC], f32, name="cxs",
                                        tag="cxs")
                    nc.vector.tensor_copy(out=cxs[:], in_=cx[hh][:])
                    nc.vector.reciprocal(out=cxs[DK:DK + 1, :],
                                         in_=cxs[DK:DK + 1, :])
                    nc.vector.tensor_copy(out=recB[0:1, :],
                                          in_=cxs[DK:DK + 1, :])
                    nc.vector.tensor_copy(out=recB[32:33, :],
                                          in_=cxs[DK:DK + 1, :])
                    nc.vector.stream_shuffle(out=recB[:], in_=recB[:],
                                             mask=[0] * 32)
                    nc.vector.tensor_mul(
                        out=ctxT[pair][hh * DK:(hh + 1) * DK, q0:q0 + SQC],
                        in0=cxs[0:DK, :],
                        in1=recB[:])

            def drain(gen):
                while next(gen, _SENT) is not _SENT:
                    pass

            def emit_full():
                import itertools as it
                if not loads_outside:
                    load_all()
                # serial prep head: first half of the v projection (pipelines
                # through the free sc-slot rotation), then pair0's q(sh0) + k;
                # v's second half rides chunk (0,0) 8 iterations ahead of its
                # PV consumer.
                # Serial head ordered by DMA arrival: v st0-7 (xv g0/g1 land
                # first on the sync queue), pair0 q(sh0)/k (xq/xk sh halves),
                # then v st8-15 (xv g2/g3 land ~20us in). The whole v
                # projection runs dense here; attention chunks then carry
                # only a THIN qk pump (~1 MM/batch from one global
                # generator), which fits inside the exp-paced slack with no
                # serialized drain leftovers - measured better than pumping
                # v through chunk 0 and qk at rate 4 with per-pair drains.
                for st in range(0, NSKT // 2, 2):
                    v_proj_st2(st, st + 1)
                drain(proj_qk_sh(0, "q", 0))
                drain(proj_qk_sh(0, "k", 0))
                drain(proj_qk_sh(0, "k", 1))
                for st in range(NSKT // 2, NSKT, 2):
                    v_proj_st2(st, st + 1)
                g = it.chain(proj_qk_sh(0, "q", 1), pair_proj(1),
                             pair_proj(2), pair_proj(3))
                for pair in range(3):
                    for sqc in range(NSQC):
                        attention_chunk(pair, sqc, pump=g)
                drain(g)
                # pair 3: pump the out-projection, one sq-chunk behind
                for sqc in range(NSQC):
                    g = (outproj_gen(range(4 * (sqc - 1), 4 * sqc))
                         if sqc >= 1 else None)
                    attention_chunk(3, sqc, pump=g, pump_rate=6)
                    if g is not None:
                        drain(g)
                drain(outproj_gen(range(12, 16)))

            def emit_attn_only():
                for pair in range(NPAIR):
                    for sqc in range(NSQC):
                        attention_chunk(pair, sqc)
                drain(outproj_gen(range(16)))

            def emit_attn_noproj():
                for pair in range(NPAIR):
                    for sqc in range(NSQC):
                        attention_chunk(pair, sqc)
                y_sb = y_pool.tile([P, D], f32, name="ycons", tag="y")
                nc.vector.tensor_copy(out=y_sb[:, 0:S // 16],
                                      in_=ctxT[0][:, 0:S // 16])
                nc.sync.dma_start(out=yp[0:P, :], in_=y_sb[:])

            def emit_prep_only():
                load_all()
                for st in range(0, NSKT, 2):
                    v_proj_st2(st, st + 1)
                for pair in range(NPAIR):
                    drain(pair_proj(pair))
                y_sb = y_pool.tile([P, D], f32, name="ycons", tag="y")
                nc.vector.tensor_copy(out=y_sb[:, 0:S // 16],
                                      in_=qhT[0][:, 0:S // 16])
                nc.sync.dma_start(out=yp[0:P, :], in_=y_sb[:])

            def emit_all():
                if parts == "attn":
                    emit_attn_only()
                elif parts == "attn_noproj":
                    emit_attn_noproj()
                elif parts == "prep":
                    emit_prep_only()
                else:
                    emit_full()

            import contextlib as _ctl
            if parts == "attn":
                for pc in range(4):
                    nc.gpsimd.dma_start(out=wo_sb[pc][:],
                                        in_=woT[pc * P:(pc + 1) * P, :])
            if parts in ("attn", "attn_noproj"):
                # one-time setup outside the timing loop
                for t in qhT + khT + ctxT:
                    nc.vector.memset(t[:], 0.0)
                for t in vh:
                    nc.vector.memset(t[:], 1.0)
            if loads_outside:
                load_all()
            loop_cm = tc.For_i(0, loop_n, 1) if loop_n else _ctl.nullcontext()
            with loop_cm:
                emit_all()

    nc.compile()
    return nc


def _get_module(loop_n=None):
    key = ("nc", loop_n)
    if key not in _CACHE:
        _CACHE[key] = _build_module(loop_n=loop_n)
    return _CACHE[key]


def _make_in_maps(q, k, v, Wq, bq, Wk, bk, Wv, bv, Wo):
    import ml_dtypes
    bf16 = ml_dtypes.bfloat16

    def T(a):
        # bf16 cast first (cheap, contiguous), then transpose-copy in bf16
        return np.ascontiguousarray(a.astype(bf16).T)

    qT = [T(q[b]) for b in range(B)]
    kT = [T(k[b]) for b in range(B)]
    vT = [T(v[b]) for b in range(B)]
    in_maps = []
    for c in range(NCORES):
        b, g = c // 2, c % 2
        eg = slice(g * EG, (g + 1) * EG)
        in_maps.append({
            "xqT": qT[b],
            "xkT": kT[b],
            "xvT": vT[b],
            "wqT": T(Wq[eg]),
            "wkT": T(Wk[eg]),
            "wvT": T(Wv[eg]),
            "woT": T(Wo[:, eg]),
            "bq": np.ascontiguousarray(bq[eg], dtype=np.float32),
            "bk": np.ascontiguousarray(bk[eg], dtype=np.float32),
            "bv": np.ascontiguousarray(bv[eg], dtype=np.float32),
        })
    return in_maps


def kernel(q, k, v, mask, Wq, bq, Wk, bk, Wv, bv, Wo, bo):
    from concourse.bass_utils import run_bass_kernel_spmd

    q = np.asarray(q, dtype=np.float32)
    k = np.asarray(k, dtype=np.float32)
    v = np.asarray(v, dtype=np.float32)
    Wq, Wk, Wv, Wo = (np.asarray(a, dtype=np.float32) for a in (Wq, Wk, Wv, Wo))
    bq, bk, bv, bo = (np.asarray(a, dtype=np.float32) for a in (bq, bk, bv, bo))

    nc = _get_module()
    in_maps = _make_in_maps(q, k, v, Wq, bq, Wk, bk, Wv, bv, Wo)
    res = run_bass_kernel_spmd(nc, in_maps, core_ids=list(range(NCORES)))

    out = np.empty((B, S, D), dtype=np.float32)
    for b in range(B):
        out[b] = res.results[2 * b]["yp"] + res.results[2 * b + 1]["yp"] + bo
    return out

